# revision 17
# baseline (speedup 1.0000x reference)
"""Trainium2 Bass kernel for a debiased GRU cell.

Computation (per batch row):
    r   = sigmoid(W_r @ [x; h] + b_r)
    u   = sigmoid(W_u @ [x; h] + b_u)
    hh  = tanh(W_h @ [x_int; r*h] + b_h)
    s   = score * u
    out = (1 - s) * hh + s * h

Strategy: data-parallel over 8 cores (8192 rows each), feature-major
on-chip layout ([H, batch]) so activations never need an on-chip
transpose. Precision plan (output tolerance is 2e-2):
  - gate matmuls (75% of PE work) run in fp8e4m3 with DoubleRow perf
    mode (2 K-planes per pass, ~1.4x the bf16 rate). Gate weights are
    pre-scaled by 32 on the host so they clear the fp8 subnormal range;
    the 1/32 is folded into the sigmoid's scale operand.
  - the h_hat matmul keeps bf16 operands (its error feeds the output
    directly, fp8 there would blow the budget).
  - elementwise chain + output run in bf16 (2x DVE rate, half the DMA).
Host packs/casts all inputs; host unpacks the bf16 output.
"""

import os

import numpy as np

import concourse.bacc as bacc
import concourse.bass as bass
import concourse.mybir as mybir
import concourse.tile as tile
from concourse.bass_utils import run_bass_kernel_spmd

B = 65536
I = 256
H = 256
NCORES = 8
BC = B // NCORES  # rows per core
NB = 512          # batch columns per block (PSUM bank = 512 fp32)
NBLK = BC // NB   # 16
FP32 = mybir.dt.float32
BF16 = mybir.dt.bfloat16
FP8 = mybir.dt.float8e4
AF = mybir.ActivationFunctionType
DR = mybir.MatmulPerfMode.DoubleRow
WSCALE = 32.0  # gate-weight prescale to clear the fp8e4m3 subnormal range

_NC_CACHE = {}


def _build_nc(reps=1, loop=None, gate_fp8=True, rh_fp8=False, a_pool=False,
              pg_bufs=6, ph_bufs=2, in_bufs=3, work_bufs=3,
              out_queue="scalar", h_queue="sync", split_loads=True, group=2):
    nc = bacc.Bacc(
        "TRN2",
        target_bir_lowering=False,
        debug=False,
        enable_asserts=False,
    )

    GDT = FP8 if gate_fp8 else BF16
    x8 = nc.dram_tensor("x8", [2 * I, BC], GDT, kind="ExternalInput")
    h8 = nc.dram_tensor("h8", [H, BC], GDT, kind="ExternalInput")
    xi = nc.dram_tensor("xi", [I, BC], BF16, kind="ExternalInput")
    hb = nc.dram_tensor("hb", [H, BC], BF16, kind="ExternalInput")
    sc = nc.dram_tensor("sc", [NBLK, 1, NB], BF16, kind="ExternalInput")
    # gate weights: [p, gi*6 + c*2 + plane, m] (fp8, x WSCALE) for DoubleRow,
    # or [p, gi*6 + k, m] (bf16, k = 6 K-chunks) for the bf16 fallback
    wg = nc.dram_tensor("wg", [128, 24, 128], GDT, kind="ExternalInput")
    # h_hat weights: x_int chunks in bf16, rh chunks fp8 (DoubleRow) or bf16.
    # Both carry the WSCALE prescale (folded out in the tanh's scale) so the
    # psum scale is uniform across chunks.
    whx = nc.dram_tensor("whx", [128, 4, 128], BF16, kind="ExternalInput")
    RDT = FP8 if rh_fp8 else BF16
    whr = nc.dram_tensor("whr", [128, 4, 128], RDT, kind="ExternalInput")
    bg = nc.dram_tensor("bg", [128, 4], FP32, kind="ExternalInput")
    bh = nc.dram_tensor("bh", [128, 2], FP32, kind="ExternalInput")
    outT = nc.dram_tensor("outT", [H, BC], BF16, kind="ExternalOutput")

    # [blk, partition, k-chunk, col] — DMA at `group`-block granularity
    GNB = group * NB
    x8r = x8.rearrange("(k p) (b n) -> b p k n", p=128, n=GNB)
    h8r = h8.rearrange("(k p) (b n) -> b p k n", p=128, n=GNB)
    xir = xi.rearrange("(k p) (b n) -> b p k n", p=128, n=GNB)
    hbr = hb.rearrange("(k p) (b n) -> b p k n", p=128, n=GNB)
    scr = sc.rearrange("b o n -> b o n") if group == 1 else \
        sc.rearrange("(g j) o n -> g o (j n)", j=group)
    outTr = outT.rearrange("(m p) (b n) -> b p m n", p=128, n=GNB)

    with tile.TileContext(nc) as tc:
        with (
            tc.tile_pool(name="const", bufs=1) as cpool,
            tc.tile_pool(name="xin", bufs=in_bufs) as xpool,
            tc.tile_pool(name="hin", bufs=in_bufs) as hpool,
            tc.tile_pool(name="sin", bufs=in_bufs) as spool,
            tc.tile_pool(name="gates", bufs=work_bufs) as gpool,
            tc.tile_pool(name="work", bufs=work_bufs) as wpool,
            tc.tile_pool(name="outp", bufs=work_bufs) as opool,
            tc.tile_pool(name="psg", bufs=pg_bufs, space=bass.MemorySpace.PSUM) as pgpool,
            tc.tile_pool(name="psh", bufs=ph_bufs, space=bass.MemorySpace.PSUM) as phpool,
        ):
            # Gate weights split per gate-half so the first gate chain only
            # waits on its own slice, not the full weight load.
            wg_sb = cpool.tile([128, 24, 128], GDT)
            for gi in range(4):
                nc.sync.dma_start(wg_sb[:, gi * 6:(gi + 1) * 6, :],
                                  wg[:, gi * 6:(gi + 1) * 6, :])
            bg_sb = cpool.tile([128, 4], FP32)
            nc.sync.dma_start(bg_sb[:], bg[:])
            whx_sb = cpool.tile([128, 4, 128], BF16)
            nc.sync.dma_start(whx_sb[:], whx[:])
            whr_sb = cpool.tile([128, 4, 128], RDT)
            nc.sync.dma_start(whr_sb[:], whr[:])
            bh_sb = cpool.tile([128, 2], FP32)
            nc.sync.dma_start(bh_sb[:], bh[:])

            ENG = {"sync": nc.sync, "scalar": nc.scalar, "vector": nc.vector,
                   "pool": nc.gpsimd}
            h_eng = ENG[h_queue]
            o_eng = ENG[out_queue]

            def load_group(g):
                """DMA the inputs for blocks [g*group, (g+1)*group) in one
                burst each, plus the group-wide output staging tile."""
                xt8 = xpool.tile([128, 4, GNB], GDT, tag="xt8")
                if split_loads:
                    nc.sync.dma_start(xt8[:, 0:2, :], x8r[g][:, 0:2, :])
                    nc.sync.dma_start(xt8[:, 2:4, :], x8r[g][:, 2:4, :])
                else:
                    nc.sync.dma_start(xt8[:], x8r[g])
                ht8 = hpool.tile([128, 2, GNB], GDT, tag="ht8")
                h_eng.dma_start(ht8[:], h8r[g])
                xib = xpool.tile([128, 2, GNB], BF16, tag="xib")
                nc.sync.dma_start(xib[:], xir[g])
                htb = hpool.tile([128, 2, GNB], BF16, tag="htb")
                h_eng.dma_start(htb[:], hbr[g])
                srow = spool.tile([1, GNB], BF16, tag="srow")
                nc.sync.dma_start(srow[:], scr[g])
                sbc = spool.tile([128, 2, GNB], BF16, tag="sbc")
                nc.gpsimd.partition_broadcast(sbc[:, 0, :], srow[:])
                nc.gpsimd.partition_broadcast(sbc[:, 1, :], srow[:])
                og = opool.tile([128, 2, GNB], BF16, tag="o")
                return dict(g=g, xt8=xt8, ht8=ht8, xib=xib, htb=htb,
                            sbc=sbc, og=og)

            def emit_gates(grp, j):
                """Gate matmuls + sigmoids + r*h for sub-block j of a group."""
                b = grp["g"] * group + j
                js = slice(j * NB, (j + 1) * NB)
                xt8 = grp["xt8"]
                ht8 = grp["ht8"]
                htb = grp["htb"]

                pgs = [pgpool.tile([128, NB], FP32, tag="pg", name=f"pg{b}_{i}")
                       for i in range(4)]
                for gi in range(4):  # r0, r1, u0, u1
                    if gate_fp8:
                        chunks = [xt8[:, 0:2, js], xt8[:, 2:4, js],
                                  ht8[:, :, js]]
                        for c, rhs in enumerate(chunks):
                            nc.tensor.matmul(
                                pgs[gi][:],
                                wg_sb[:, gi * 6 + 2 * c:gi * 6 + 2 * c + 2, :],
                                rhs,
                                start=(c == 0),
                                stop=(c == 2),
                                perf_mode=DR,
                            )
                    else:
                        for k in range(6):
                            act = xt8[:, k, js] if k < 4 else ht8[:, k - 4, js]
                            nc.tensor.matmul(
                                pgs[gi][:],
                                wg_sb[:, gi * 6 + k, :],
                                act,
                                start=(k == 0),
                                stop=(k == 5),
                            )
                r = gpool.tile([128, 2, NB], BF16, tag="r")
                u = gpool.tile([128, 2, NB], BF16, tag="u")
                inv = 1.0 / WSCALE if gate_fp8 else 1.0
                for m in range(2):
                    nc.scalar.activation(
                        r[:, m, :], pgs[m][:],
                        AF.Sigmoid, bias=bg_sb[:, m:m + 1], scale=inv,
                    )
                    nc.scalar.activation(
                        u[:, m, :], pgs[2 + m][:],
                        AF.Sigmoid, bias=bg_sb[:, 2 + m:3 + m], scale=inv,
                    )
                rh = wpool.tile([128, 2, NB], RDT, tag="rh")
                nc.vector.tensor_mul(rh[:], r[:], htb[:, :, js])
                # e2 = score*u and A = h*e2 only depend on the gate phase, so
                # they run here, off the post-tanh critical tail. A runs on
                # the otherwise-idle GPSIMD to unload the DVE.
                e2 = wpool.tile([128, 2, NB], BF16, tag="e2")
                nc.vector.tensor_mul(e2[:], u[:], grp["sbc"][:, :, js])
                A = wpool.tile([128, 2, NB], BF16, tag="A")
                a_eng = nc.gpsimd if a_pool else nc.vector
                a_eng.tensor_mul(A[:], htb[:, :, js], e2[:])
                return dict(b=b, j=j, grp=grp, rh=rh, e2=e2, A=A)

            def emit_h(st):
                """h_hat matmul + tanh + final combine + store for block b."""
                b = st["b"]
                j = st["j"]
                js = slice(j * NB, (j + 1) * NB)
                xib = st["grp"]["xib"]
                phs = [phpool.tile([128, NB], FP32, tag="ph", name=f"ph{b}_{i}")
                       for i in range(2)]
                for m in range(2):
                    for k in range(2):
                        nc.tensor.matmul(
                            phs[m][:],
                            whx_sb[:, m * 2 + k, :],
                            xib[:, k, js],
                            start=(k == 0),
                            stop=False,
                        )
                    if rh_fp8:
                        nc.tensor.matmul(
                            phs[m][:],
                            whr_sb[:, 2 * m:2 * m + 2, :],
                            st["rh"][:],
                            start=False,
                            stop=True,
                            perf_mode=DR,
                        )
                    else:
                        for k in range(2):
                            nc.tensor.matmul(
                                phs[m][:],
                                whr_sb[:, 2 * m + k, :],
                                st["rh"][:, k, :],
                                start=False,
                                stop=(k == 1),
                            )
                hhat = wpool.tile([128, 2, NB], BF16, tag="hhat")
                hsc = 1.0 / WSCALE if rh_fp8 else 1.0
                for m in range(2):
                    nc.scalar.activation(
                        hhat[:, m, :], phs[m][:],
                        AF.Tanh, bias=bh_sb[:, m:m + 1], scale=hsc,
                    )
                # out = A - (e2-1)*hh  ==  hh + e2*(h - hh), with A = h*e2
                C = wpool.tile([128, 2, NB], BF16, tag="C")
                nc.vector.scalar_tensor_tensor(
                    C[:], st["e2"][:], 1.0, hhat[:],
                    op0=mybir.AluOpType.subtract, op1=mybir.AluOpType.mult,
                )
                og = st["grp"]["og"]
                nc.vector.tensor_sub(og[:, :, js], st["A"][:], C[:])
                if j == group - 1:
                    # store off the SP/ACT rings so it doesn't serialize with
                    # input-load dispatch or activation dispatch
                    o_eng.dma_start(outTr[st["grp"]["g"]], og[:])

            # Software-pipelined emission: block b's h-chain is emitted after
            # block b+1's gate matmuls so the PE never waits on the r*h
            # elementwise product.
            def emit_pass():
                prev = None
                for _rep in range(reps):
                    for g in range(NBLK // group):
                        grp = load_group(g)
                        for j in range(group):
                            st = emit_gates(grp, j)
                            if prev is not None:
                                emit_h(prev)
                            prev = st
                emit_h(prev)

            if loop is None:
                emit_pass()
            else:
                # bench-only: repeat the whole pass `loop` times inside one
                # NEFF execution for slope-based timing.
                with tc.For_i(0, loop, 1):
                    emit_pass()

    nc.compile()
    return nc


def _get_nc():
    if "nc" not in _NC_CACHE:
        _NC_CACHE["nc"] = _build_nc()
    return _NC_CACHE["nc"]


def _pack_weights(W_r, W_u, W_h, b_r, b_u, b_h, gate_fp8=True, rh_fp8=True):
    np8 = mybir.dt.np(FP8)
    npbf = mybir.dt.np(BF16)
    wg = np.empty((128, 24, 128), np.float32)
    for gi in range(4):
        W = W_r if gi < 2 else W_u
        m = gi % 2
        for k in range(6):
            # fp8 DoubleRow: slot gi*6 + c*2 + plane == gi*6 + k with
            # k = 2c + plane covering K rows [128k, 128k+128) — identical
            # packing for the bf16 fallback.
            wg[:, gi * 6 + k, :] = W[m * 128:(m + 1) * 128,
                                     k * 128:(k + 1) * 128].T
    if gate_fp8:
        wg = (wg * WSCALE).astype(np8)
    else:
        wg = wg.astype(npbf)
    hscale = WSCALE if rh_fp8 else 1.0
    whx = np.empty((128, 4, 128), np.float32)
    whr = np.empty((128, 4, 128), np.float32)
    for m in range(2):
        for k in range(2):
            whx[:, m * 2 + k, :] = W_h[m * 128:(m + 1) * 128,
                                       k * 128:(k + 1) * 128].T * hscale
            whr[:, m * 2 + k, :] = W_h[m * 128:(m + 1) * 128,
                                       (2 + k) * 128:(3 + k) * 128].T * hscale
    whx = whx.astype(npbf)
    whr = whr.astype(np8 if rh_fp8 else npbf)
    bg = np.stack([b_r[:128], b_r[128:], b_u[:128], b_u[128:]], axis=1)
    bh = np.stack([b_h[:128], b_h[128:]], axis=1)
    return (np.ascontiguousarray(wg), np.ascontiguousarray(whx),
            np.ascontiguousarray(whr),
            np.ascontiguousarray(bg), np.ascontiguousarray(bh))


def _make_in_maps(inputs, h_prev, attention_score, W_r, b_r, W_u, b_u,
                  W_h, b_h, gate_fp8=True, rh_fp8=False):
    np8 = mybir.dt.np(FP8)
    npbf = mybir.dt.np(BF16)
    gdt = np8 if gate_fp8 else npbf
    inputs = np.asarray(inputs, np.float32)
    h_prev = np.asarray(h_prev, np.float32)
    attention_score = np.asarray(attention_score, np.float32)
    wg, whx, whr, bg, bh = _pack_weights(
        np.asarray(W_r, np.float32), np.asarray(W_u, np.float32),
        np.asarray(W_h, np.float32), np.asarray(b_r, np.float32),
        np.asarray(b_u, np.float32), np.asarray(b_h, np.float32),
        gate_fp8=gate_fp8, rh_fp8=rh_fp8,
    )
    in_maps = []
    for c in range(NCORES):
        sl = slice(c * BC, (c + 1) * BC)
        xT = np.ascontiguousarray(inputs[sl].T)
        hT = np.ascontiguousarray(h_prev[sl].T)
        in_maps.append({
            "x8": xT.astype(gdt),
            "h8": hT.astype(gdt),
            "xi": np.ascontiguousarray(xT[:I]).astype(npbf),
            "hb": hT.astype(npbf),
            "sc": np.ascontiguousarray(
                attention_score[sl].reshape(NBLK, 1, NB)).astype(npbf),
            "wg": wg, "whx": whx, "whr": whr, "bg": bg, "bh": bh,
        })
    return in_maps


def _run(in_maps, trace=False, **kwargs):
    try:
        return run_bass_kernel_spmd(
            _get_nc(), in_maps, core_ids=list(range(NCORES)), trace=trace, **kwargs
        )
    except ModuleNotFoundError:
        # A global BASS_TRACE=1 enables the NTFF trace path, which needs
        # antenv.axon_hooks; on images without it, retry untraced.
        had = os.environ.get("BASS_NEVER_TRACE")
        os.environ["BASS_NEVER_TRACE"] = "1"
        try:
            return run_bass_kernel_spmd(
                _get_nc(), in_maps, core_ids=list(range(NCORES)), trace=False,
                **kwargs
            )
        finally:
            if had is None:
                del os.environ["BASS_NEVER_TRACE"]
            else:
                os.environ["BASS_NEVER_TRACE"] = had


def _gather(results):
    out = np.empty((B, H), np.float32)
    for c in range(NCORES):
        out[c * BC:(c + 1) * BC] = results[c]["outT"].T.astype(np.float32)
    return out


def kernel(**inputs):
    res = _run(_make_in_maps(**inputs), trace=False)
    return _gather(res.results)


# revision 23
# speedup vs baseline: 1.0551x; 1.0551x over previous
"""Trainium2 Bass kernel for a debiased GRU cell.

Computation (per batch row):
    r   = sigmoid(W_r @ [x; h] + b_r)
    u   = sigmoid(W_u @ [x; h] + b_u)
    hh  = tanh(W_h @ [x_int; r*h] + b_h)
    s   = score * u
    out = (1 - s) * hh + s * h

Strategy: data-parallel over 8 cores (8192 rows each), feature-major
on-chip layout ([H, batch]) so activations never need an on-chip
transpose. Precision plan (output tolerance is 2e-2):
  - gate matmuls (75% of PE work) run in fp8e4m3 with DoubleRow perf
    mode (2 K-planes per pass, ~1.4x the bf16 rate). Gate weights are
    pre-scaled by 32 on the host so they clear the fp8 subnormal range;
    the 1/32 is folded into the sigmoid's scale operand.
  - the h_hat matmul keeps bf16 operands (its error feeds the output
    directly, fp8 there would blow the budget).
  - elementwise chain + output run in bf16 (2x DVE rate, half the DMA).
Host packs/casts all inputs; host unpacks the bf16 output.
"""

import os

import numpy as np

import concourse.bacc as bacc
import concourse.bass as bass
import concourse.mybir as mybir
import concourse.tile as tile
from concourse.bass_utils import run_bass_kernel_spmd

B = 65536
I = 256
H = 256
NCORES = 8
BC = B // NCORES  # rows per core
NB = 512          # batch columns per block (PSUM bank = 512 fp32)
NBLK = BC // NB   # 16
FP32 = mybir.dt.float32
BF16 = mybir.dt.bfloat16
FP8 = mybir.dt.float8e4
AF = mybir.ActivationFunctionType
DR = mybir.MatmulPerfMode.DoubleRow
WSCALE = 32.0  # gate-weight prescale to clear the fp8e4m3 subnormal range

_NC_CACHE = {}


def _build_nc(reps=1, loop=None, gate_fp8=True, rh_fp8=False, a_pool=False,
              swi=True,
              pg_bufs=6, ph_bufs=2, in_bufs=3, work_bufs=3,
              out_queue="scalar", h_queue="sync", split_loads=True, group=2):
    nc = bacc.Bacc(
        "TRN2",
        target_bir_lowering=False,
        debug=False,
        enable_asserts=False,
    )

    GDT = FP8 if gate_fp8 else BF16
    x8 = nc.dram_tensor("x8", [2 * I, BC], GDT, kind="ExternalInput")
    h8 = nc.dram_tensor("h8", [H, BC], GDT, kind="ExternalInput")
    xi = nc.dram_tensor("xi", [I, BC], BF16, kind="ExternalInput")
    hb = nc.dram_tensor("hb", [H, BC], BF16, kind="ExternalInput")
    sc = nc.dram_tensor("sc", [NBLK, 1, NB], BF16, kind="ExternalInput")
    # gate weights: [p, gi*6 + c*2 + plane, m] (fp8, x WSCALE) for DoubleRow,
    # or [p, gi*6 + k, m] (bf16, k = 6 K-chunks) for the bf16 fallback
    wg = nc.dram_tensor("wg", [128, 24, 128], GDT, kind="ExternalInput")
    # h_hat weights: x_int chunks in bf16, rh chunks fp8 (DoubleRow) or bf16.
    # Both carry the WSCALE prescale (folded out in the tanh's scale) so the
    # psum scale is uniform across chunks.
    whx = nc.dram_tensor("whx", [128, 4, 128], BF16, kind="ExternalInput")
    RDT = FP8 if rh_fp8 else BF16
    whr = nc.dram_tensor("whr", [128, 4, 128], RDT, kind="ExternalInput")
    bg = nc.dram_tensor("bg", [128, 4], FP32, kind="ExternalInput")
    bh = nc.dram_tensor("bh", [128, 2], FP32, kind="ExternalInput")
    outT = nc.dram_tensor("outT", [H, BC], BF16, kind="ExternalOutput")

    # [blk, partition, k-chunk, col] — DMA at `group`-block granularity
    GNB = group * NB
    x8r = x8.rearrange("(k p) (b n) -> b p k n", p=128, n=GNB)
    h8r = h8.rearrange("(k p) (b n) -> b p k n", p=128, n=GNB)
    xir = xi.rearrange("(k p) (b n) -> b p k n", p=128, n=GNB)
    hbr = hb.rearrange("(k p) (b n) -> b p k n", p=128, n=GNB)
    scr = sc.rearrange("b o n -> b o n") if group == 1 else \
        sc.rearrange("(g j) o n -> g o (j n)", j=group)
    outTr = outT.rearrange("(m p) (b n) -> b p m n", p=128, n=GNB)

    with tile.TileContext(nc) as tc:
        with (
            tc.tile_pool(name="const", bufs=1) as cpool,
            tc.tile_pool(name="xin", bufs=in_bufs) as xpool,
            tc.tile_pool(name="hin", bufs=in_bufs) as hpool,
            tc.tile_pool(name="sin", bufs=in_bufs) as spool,
            tc.tile_pool(name="gates", bufs=work_bufs) as gpool,
            tc.tile_pool(name="work", bufs=work_bufs) as wpool,
            tc.tile_pool(name="outp", bufs=work_bufs) as opool,
            tc.tile_pool(name="psg", bufs=pg_bufs, space=bass.MemorySpace.PSUM) as pgpool,
            tc.tile_pool(name="psh", bufs=ph_bufs, space=bass.MemorySpace.PSUM) as phpool,
        ):
            # Gate weights split per gate-half so the first gate chain only
            # waits on its own slice, not the full weight load.
            wg_sb = cpool.tile([128, 24, 128], GDT)
            for gi in range(4):
                nc.sync.dma_start(wg_sb[:, gi * 6:(gi + 1) * 6, :],
                                  wg[:, gi * 6:(gi + 1) * 6, :])
            bg_sb = cpool.tile([128, 4], FP32)
            nc.sync.dma_start(bg_sb[:], bg[:])
            whx_sb = cpool.tile([128, 4, 128], BF16)
            nc.sync.dma_start(whx_sb[:], whx[:])
            whr_sb = cpool.tile([128, 4, 128], RDT)
            nc.sync.dma_start(whr_sb[:], whr[:])
            bh_sb = cpool.tile([128, 2], FP32)
            nc.sync.dma_start(bh_sb[:], bh[:])

            ENG = {"sync": nc.sync, "scalar": nc.scalar, "vector": nc.vector,
                   "pool": nc.gpsimd}
            h_eng = ENG[h_queue]
            o_eng = ENG[out_queue]

            def load_group(g):
                """DMA the inputs for blocks [g*group, (g+1)*group) in one
                burst each, plus the group-wide output staging tile."""
                xt8 = xpool.tile([128, 4, GNB], GDT, tag="xt8")
                if split_loads:
                    nc.sync.dma_start(xt8[:, 0:2, :], x8r[g][:, 0:2, :])
                    nc.sync.dma_start(xt8[:, 2:4, :], x8r[g][:, 2:4, :])
                else:
                    nc.sync.dma_start(xt8[:], x8r[g])
                ht8 = hpool.tile([128, 2, GNB], GDT, tag="ht8")
                h_eng.dma_start(ht8[:], h8r[g])
                xib = xpool.tile([128, 2, GNB], BF16, tag="xib")
                nc.sync.dma_start(xib[:], xir[g])
                htb = hpool.tile([128, 2, GNB], BF16, tag="htb")
                h_eng.dma_start(htb[:], hbr[g])
                srow = spool.tile([1, GNB], BF16, tag="srow")
                nc.sync.dma_start(srow[:], scr[g])
                sbc = spool.tile([128, 2, GNB], BF16, tag="sbc")
                nc.gpsimd.partition_broadcast(sbc[:, 0, :], srow[:])
                nc.gpsimd.partition_broadcast(sbc[:, 1, :], srow[:])
                og = opool.tile([128, 2, GNB], BF16, tag="o")
                return dict(g=g, xt8=xt8, ht8=ht8, xib=xib, htb=htb,
                            sbc=sbc, og=og)

            def emit_gates(grp, j):
                """Gate matmuls + sigmoids + r*h for sub-block j of a group."""
                b = grp["g"] * group + j
                js = slice(j * NB, (j + 1) * NB)
                xt8 = grp["xt8"]
                ht8 = grp["ht8"]
                htb = grp["htb"]

                pgs = [pgpool.tile([128, NB], FP32, tag="pg", name=f"pg{b}_{i}")
                       for i in range(4)]
                gmode = mybir.MatmulPerfMode.DoubleRowSwInterleave if swi else DR
                for gi in range(4):  # r0, r1, u0, u1
                    if gate_fp8:
                        chunks = [xt8[:, 0:2, js], xt8[:, 2:4, js],
                                  ht8[:, :, js]]
                        for c, rhs in enumerate(chunks):
                            nc.tensor.matmul(
                                pgs[gi][:],
                                wg_sb[:, gi * 6 + 2 * c:gi * 6 + 2 * c + 2, :],
                                rhs,
                                start=(c == 0),
                                stop=(c == 2),
                                perf_mode=gmode,
                            )
                    else:
                        for k in range(6):
                            act = xt8[:, k, js] if k < 4 else ht8[:, k - 4, js]
                            nc.tensor.matmul(
                                pgs[gi][:],
                                wg_sb[:, gi * 6 + k, :],
                                act,
                                start=(k == 0),
                                stop=(k == 5),
                            )
                r = gpool.tile([128, 2, NB], BF16, tag="r")
                u = gpool.tile([128, 2, NB], BF16, tag="u")
                inv = 1.0 / WSCALE if gate_fp8 else 1.0
                for m in range(2):
                    nc.scalar.activation(
                        r[:, m, :], pgs[m][:],
                        AF.Sigmoid, bias=bg_sb[:, m:m + 1], scale=inv,
                    )
                    nc.scalar.activation(
                        u[:, m, :], pgs[2 + m][:],
                        AF.Sigmoid, bias=bg_sb[:, 2 + m:3 + m], scale=inv,
                    )
                rh = wpool.tile([128, 2, NB], RDT, tag="rh")
                nc.vector.tensor_mul(rh[:], r[:], htb[:, :, js])
                # e2 = score*u and A = h*e2 only depend on the gate phase, so
                # they run here, off the post-tanh critical tail. A runs on
                # the otherwise-idle GPSIMD to unload the DVE.
                e2 = wpool.tile([128, 2, NB], BF16, tag="e2")
                nc.vector.tensor_mul(e2[:], u[:], grp["sbc"][:, :, js])
                A = wpool.tile([128, 2, NB], BF16, tag="A")
                a_eng = nc.gpsimd if a_pool else nc.vector
                a_eng.tensor_mul(A[:], htb[:, :, js], e2[:])
                return dict(b=b, j=j, grp=grp, rh=rh, e2=e2, A=A)

            def emit_h(st):
                """h_hat matmul + tanh + final combine + store for block b."""
                b = st["b"]
                j = st["j"]
                js = slice(j * NB, (j + 1) * NB)
                xib = st["grp"]["xib"]
                phs = [phpool.tile([128, NB], FP32, tag="ph", name=f"ph{b}_{i}")
                       for i in range(2)]
                for m in range(2):
                    for k in range(2):
                        nc.tensor.matmul(
                            phs[m][:],
                            whx_sb[:, m * 2 + k, :],
                            xib[:, k, js],
                            start=(k == 0),
                            stop=False,
                        )
                    if rh_fp8:
                        nc.tensor.matmul(
                            phs[m][:],
                            whr_sb[:, 2 * m:2 * m + 2, :],
                            st["rh"][:],
                            start=False,
                            stop=True,
                            perf_mode=DR,
                        )
                    else:
                        for k in range(2):
                            nc.tensor.matmul(
                                phs[m][:],
                                whr_sb[:, 2 * m + k, :],
                                st["rh"][:, k, :],
                                start=False,
                                stop=(k == 1),
                            )
                hhat = wpool.tile([128, 2, NB], BF16, tag="hhat")
                hsc = 1.0 / WSCALE if rh_fp8 else 1.0
                for m in range(2):
                    nc.scalar.activation(
                        hhat[:, m, :], phs[m][:],
                        AF.Tanh, bias=bh_sb[:, m:m + 1], scale=hsc,
                    )
                # out = A - (e2-1)*hh  ==  hh + e2*(h - hh), with A = h*e2
                C = wpool.tile([128, 2, NB], BF16, tag="C")
                nc.vector.scalar_tensor_tensor(
                    C[:], st["e2"][:], 1.0, hhat[:],
                    op0=mybir.AluOpType.subtract, op1=mybir.AluOpType.mult,
                )
                og = st["grp"]["og"]
                nc.vector.tensor_sub(og[:, :, js], st["A"][:], C[:])
                if j == group - 1:
                    # store off the SP/ACT rings so it doesn't serialize with
                    # input-load dispatch or activation dispatch
                    o_eng.dma_start(outTr[st["grp"]["g"]], og[:])

            # Software-pipelined emission: block b's h-chain is emitted after
            # block b+1's gate matmuls so the PE never waits on the r*h
            # elementwise product.
            def emit_pass():
                prev = None
                for _rep in range(reps):
                    for g in range(NBLK // group):
                        grp = load_group(g)
                        for j in range(group):
                            st = emit_gates(grp, j)
                            if prev is not None:
                                emit_h(prev)
                            prev = st
                emit_h(prev)

            if loop is None:
                emit_pass()
            else:
                # bench-only: repeat the whole pass `loop` times inside one
                # NEFF execution for slope-based timing.
                with tc.For_i(0, loop, 1):
                    emit_pass()

    nc.compile()
    return nc


def _get_nc():
    if "nc" not in _NC_CACHE:
        _NC_CACHE["nc"] = _build_nc()
    return _NC_CACHE["nc"]


def _pack_weights(W_r, W_u, W_h, b_r, b_u, b_h, gate_fp8=True, rh_fp8=True,
                  swi=True):
    np8 = mybir.dt.np(FP8)
    npbf = mybir.dt.np(BF16)
    wg = np.empty((128, 24, 128), np.float32)
    for gi in range(4):
        W = W_r if gi < 2 else W_u
        m = gi % 2
        for k in range(6):
            # fp8 DoubleRow: slot gi*6 + c*2 + plane == gi*6 + k with
            # k = 2c + plane covering K rows [128k, 128k+128) — identical
            # packing for the bf16 fallback.
            wg[:, gi * 6 + k, :] = W[m * 128:(m + 1) * 128,
                                     k * 128:(k + 1) * 128].T
    if gate_fp8:
        wg = (wg * WSCALE).astype(np8)
        if swi:
            # DoubleRowSwInterleave weight layout: per chunk pair (A, B),
            # flat free order [A127, B127, A126, B126, ..., A0, B0].
            for s in range(0, 24, 2):
                A = wg[:, s, :].copy()
                Bm = wg[:, s + 1, :].copy()
                pair = np.empty((128, 256), wg.dtype)
                pair[:, 0::2] = A[:, ::-1]
                pair[:, 1::2] = Bm[:, ::-1]
                wg[:, s, :] = pair[:, :128]
                wg[:, s + 1, :] = pair[:, 128:]
    else:
        wg = wg.astype(npbf)
    hscale = WSCALE if rh_fp8 else 1.0
    whx = np.empty((128, 4, 128), np.float32)
    whr = np.empty((128, 4, 128), np.float32)
    for m in range(2):
        for k in range(2):
            whx[:, m * 2 + k, :] = W_h[m * 128:(m + 1) * 128,
                                       k * 128:(k + 1) * 128].T * hscale
            whr[:, m * 2 + k, :] = W_h[m * 128:(m + 1) * 128,
                                       (2 + k) * 128:(3 + k) * 128].T * hscale
    whx = whx.astype(npbf)
    whr = whr.astype(np8 if rh_fp8 else npbf)
    bg = np.stack([b_r[:128], b_r[128:], b_u[:128], b_u[128:]], axis=1)
    bh = np.stack([b_h[:128], b_h[128:]], axis=1)
    return (np.ascontiguousarray(wg), np.ascontiguousarray(whx),
            np.ascontiguousarray(whr),
            np.ascontiguousarray(bg), np.ascontiguousarray(bh))


def _make_in_maps(inputs, h_prev, attention_score, W_r, b_r, W_u, b_u,
                  W_h, b_h, gate_fp8=True, rh_fp8=False, swi=True):
    np8 = mybir.dt.np(FP8)
    npbf = mybir.dt.np(BF16)
    gdt = np8 if gate_fp8 else npbf
    inputs = np.asarray(inputs, np.float32)
    h_prev = np.asarray(h_prev, np.float32)
    attention_score = np.asarray(attention_score, np.float32)
    wg, whx, whr, bg, bh = _pack_weights(
        np.asarray(W_r, np.float32), np.asarray(W_u, np.float32),
        np.asarray(W_h, np.float32), np.asarray(b_r, np.float32),
        np.asarray(b_u, np.float32), np.asarray(b_h, np.float32),
        gate_fp8=gate_fp8, rh_fp8=rh_fp8, swi=swi,
    )
    in_maps = []
    for c in range(NCORES):
        sl = slice(c * BC, (c + 1) * BC)
        xT = np.ascontiguousarray(inputs[sl].T)
        hT = np.ascontiguousarray(h_prev[sl].T)
        in_maps.append({
            "x8": xT.astype(gdt),
            "h8": hT.astype(gdt),
            "xi": np.ascontiguousarray(xT[:I]).astype(npbf),
            "hb": hT.astype(npbf),
            "sc": np.ascontiguousarray(
                attention_score[sl].reshape(NBLK, 1, NB)).astype(npbf),
            "wg": wg, "whx": whx, "whr": whr, "bg": bg, "bh": bh,
        })
    return in_maps


def _run(in_maps, trace=False, **kwargs):
    try:
        return run_bass_kernel_spmd(
            _get_nc(), in_maps, core_ids=list(range(NCORES)), trace=trace, **kwargs
        )
    except ModuleNotFoundError:
        # A global BASS_TRACE=1 enables the NTFF trace path, which needs
        # antenv.axon_hooks; on images without it, retry untraced.
        had = os.environ.get("BASS_NEVER_TRACE")
        os.environ["BASS_NEVER_TRACE"] = "1"
        try:
            return run_bass_kernel_spmd(
                _get_nc(), in_maps, core_ids=list(range(NCORES)), trace=False,
                **kwargs
            )
        finally:
            if had is None:
                del os.environ["BASS_NEVER_TRACE"]
            else:
                os.environ["BASS_NEVER_TRACE"] = had


def _gather(results):
    out = np.empty((B, H), np.float32)
    for c in range(NCORES):
        out[c * BC:(c + 1) * BC] = results[c]["outT"].T.astype(np.float32)
    return out


def kernel(**inputs):
    res = _run(_make_in_maps(**inputs), trace=False)
    return _gather(res.results)


# revision 26
# speedup vs baseline: 1.2845x; 1.2175x over previous
"""Trainium2 Bass kernel for a debiased GRU cell.

Computation (per batch row):
    r   = sigmoid(W_r @ [x; h] + b_r)
    u   = sigmoid(W_u @ [x; h] + b_u)
    hh  = tanh(W_h @ [x_int; r*h] + b_h)
    s   = score * u
    out = (1 - s) * hh + s * h

Strategy: data-parallel over 8 cores (8192 rows each), feature-major
on-chip layout ([H, batch]) so activations never need an on-chip
transpose. Precision plan (output tolerance is 2e-2):
  - gate matmuls (75% of PE work) run in fp8e4m3 with DoubleRow perf
    mode (2 K-planes per pass, ~1.4x the bf16 rate). Gate weights are
    pre-scaled by 32 on the host so they clear the fp8 subnormal range;
    the 1/32 is folded into the sigmoid's scale operand.
  - the h_hat matmul keeps bf16 operands (its error feeds the output
    directly, fp8 there would blow the budget).
  - elementwise chain + output run in bf16 (2x DVE rate, half the DMA).
Host packs/casts all inputs; host unpacks the bf16 output.
"""

import os

import numpy as np

import concourse.bacc as bacc
import concourse.bass as bass
import concourse.mybir as mybir
import concourse.tile as tile
from concourse.bass_utils import run_bass_kernel_spmd

B = 65536
I = 256
H = 256
NCORES = 8
BC = B // NCORES  # rows per core
NB = 512          # batch columns per block (PSUM bank = 512 fp32)
NBLK = BC // NB   # 16
FP32 = mybir.dt.float32
BF16 = mybir.dt.bfloat16
FP8 = mybir.dt.float8e4
AF = mybir.ActivationFunctionType
DR = mybir.MatmulPerfMode.DoubleRow
WSCALE = 32.0  # gate-weight prescale to clear the fp8e4m3 subnormal range

_NC_CACHE = {}


def _build_nc(reps=1, loop=None, gate_fp8=True, rh_fp8=False, a_pool=False,
              swi=True,
              pg_bufs=6, ph_bufs=2, in_bufs=3, work_bufs=3,
              out_queue="alt", h_queue="sync", split_loads=True, group=2):
    nc = bacc.Bacc(
        "TRN2",
        target_bir_lowering=False,
        debug=False,
        enable_asserts=False,
    )

    GDT = FP8 if gate_fp8 else BF16
    x8 = nc.dram_tensor("x8", [2 * I, BC], GDT, kind="ExternalInput")
    h8 = nc.dram_tensor("h8", [H, BC], GDT, kind="ExternalInput")
    xi = nc.dram_tensor("xi", [I, BC], BF16, kind="ExternalInput")
    hb = nc.dram_tensor("hb", [H, BC], BF16, kind="ExternalInput")
    sc = nc.dram_tensor("sc", [NBLK, 1, NB], BF16, kind="ExternalInput")
    # gate weights: [p, gi*6 + c*2 + plane, m] (fp8, x WSCALE) for DoubleRow,
    # or [p, gi*6 + k, m] (bf16, k = 6 K-chunks) for the bf16 fallback
    wg = nc.dram_tensor("wg", [128, 24, 128], GDT, kind="ExternalInput")
    # h_hat weights: x_int chunks in bf16, rh chunks fp8 (DoubleRow) or bf16.
    # Both carry the WSCALE prescale (folded out in the tanh's scale) so the
    # psum scale is uniform across chunks.
    whx = nc.dram_tensor("whx", [128, 4, 128], BF16, kind="ExternalInput")
    RDT = FP8 if rh_fp8 else BF16
    whr = nc.dram_tensor("whr", [128, 4, 128], RDT, kind="ExternalInput")
    bg = nc.dram_tensor("bg", [128, 4], FP32, kind="ExternalInput")
    bh = nc.dram_tensor("bh", [128, 2], FP32, kind="ExternalInput")
    outT = nc.dram_tensor("outT", [H, BC], BF16, kind="ExternalOutput")

    # [blk, partition, k-chunk, col] — DMA at `group`-block granularity
    GNB = group * NB
    x8r = x8.rearrange("(k p) (b n) -> b p k n", p=128, n=GNB)
    h8r = h8.rearrange("(k p) (b n) -> b p k n", p=128, n=GNB)
    xir = xi.rearrange("(k p) (b n) -> b p k n", p=128, n=GNB)
    hbr = hb.rearrange("(k p) (b n) -> b p k n", p=128, n=GNB)
    scr = sc.rearrange("b o n -> b o n") if group == 1 else \
        sc.rearrange("(g j) o n -> g o (j n)", j=group)
    outTr = outT.rearrange("(m p) (b n) -> b p m n", p=128, n=GNB)

    with tile.TileContext(nc) as tc:
        with (
            tc.tile_pool(name="const", bufs=1) as cpool,
            tc.tile_pool(name="xin", bufs=in_bufs) as xpool,
            tc.tile_pool(name="hin", bufs=in_bufs) as hpool,
            tc.tile_pool(name="sin", bufs=in_bufs) as spool,
            tc.tile_pool(name="gates", bufs=work_bufs) as gpool,
            tc.tile_pool(name="work", bufs=work_bufs) as wpool,
            tc.tile_pool(name="outp", bufs=work_bufs) as opool,
            tc.tile_pool(name="psg", bufs=pg_bufs, space=bass.MemorySpace.PSUM) as pgpool,
            tc.tile_pool(name="psh", bufs=ph_bufs, space=bass.MemorySpace.PSUM) as phpool,
        ):
            # Gate weights split per gate-half so the first gate chain only
            # waits on its own slice, not the full weight load.
            wg_sb = cpool.tile([128, 24, 128], GDT)
            for gi in range(4):
                nc.sync.dma_start(wg_sb[:, gi * 6:(gi + 1) * 6, :],
                                  wg[:, gi * 6:(gi + 1) * 6, :])
            bg_sb = cpool.tile([128, 4], FP32)
            nc.sync.dma_start(bg_sb[:], bg[:])
            whx_sb = cpool.tile([128, 4, 128], BF16)
            nc.sync.dma_start(whx_sb[:], whx[:])
            whr_sb = cpool.tile([128, 4, 128], RDT)
            nc.sync.dma_start(whr_sb[:], whr[:])
            bh_sb = cpool.tile([128, 2], FP32)
            nc.sync.dma_start(bh_sb[:], bh[:])

            ENG = {"sync": nc.sync, "scalar": nc.scalar, "vector": nc.vector,
                   "pool": nc.gpsimd}
            h_eng = ENG[h_queue]

            def o_eng(g):
                # "alt" alternates stores between the ACT and SP HWDGE rings
                # so neither sequencer eats the full dispatch cost
                if out_queue == "alt":
                    return nc.scalar if g % 2 == 0 else nc.sync
                return ENG[out_queue]

            def load_group(g):
                """DMA the inputs for blocks [g*group, (g+1)*group) in one
                burst each, plus the group-wide output staging tile."""
                xt8 = xpool.tile([128, 4, GNB], GDT, tag="xt8")
                if split_loads:
                    nc.sync.dma_start(xt8[:, 0:2, :], x8r[g][:, 0:2, :])
                    nc.sync.dma_start(xt8[:, 2:4, :], x8r[g][:, 2:4, :])
                else:
                    nc.sync.dma_start(xt8[:], x8r[g])
                ht8 = hpool.tile([128, 2, GNB], GDT, tag="ht8")
                h_eng.dma_start(ht8[:], h8r[g])
                xib = xpool.tile([128, 2, GNB], BF16, tag="xib")
                nc.sync.dma_start(xib[:], xir[g])
                htb = hpool.tile([128, 2, GNB], BF16, tag="htb")
                h_eng.dma_start(htb[:], hbr[g])
                srow = spool.tile([1, GNB], BF16, tag="srow")
                nc.sync.dma_start(srow[:], scr[g])
                sbc = spool.tile([128, 2, GNB], BF16, tag="sbc")
                nc.gpsimd.partition_broadcast(sbc[:, 0, :], srow[:])
                nc.gpsimd.partition_broadcast(sbc[:, 1, :], srow[:])
                og = opool.tile([128, 2, GNB], BF16, tag="o")
                return dict(g=g, xt8=xt8, ht8=ht8, xib=xib, htb=htb,
                            sbc=sbc, og=og)

            def emit_gates(grp, j):
                """Gate matmuls + sigmoids + r*h for sub-block j of a group."""
                b = grp["g"] * group + j
                js = slice(j * NB, (j + 1) * NB)
                xt8 = grp["xt8"]
                ht8 = grp["ht8"]
                htb = grp["htb"]

                pgs = [pgpool.tile([128, NB], FP32, tag="pg", name=f"pg{b}_{i}")
                       for i in range(4)]
                gmode = mybir.MatmulPerfMode.DoubleRowSwInterleave if swi else DR
                for gi in range(4):  # r0, r1, u0, u1
                    if gate_fp8:
                        chunks = [xt8[:, 0:2, js], xt8[:, 2:4, js],
                                  ht8[:, :, js]]
                        for c, rhs in enumerate(chunks):
                            nc.tensor.matmul(
                                pgs[gi][:],
                                wg_sb[:, gi * 6 + 2 * c:gi * 6 + 2 * c + 2, :],
                                rhs,
                                start=(c == 0),
                                stop=(c == 2),
                                perf_mode=gmode,
                            )
                    else:
                        for k in range(6):
                            act = xt8[:, k, js] if k < 4 else ht8[:, k - 4, js]
                            nc.tensor.matmul(
                                pgs[gi][:],
                                wg_sb[:, gi * 6 + k, :],
                                act,
                                start=(k == 0),
                                stop=(k == 5),
                            )
                r = gpool.tile([128, 2, NB], BF16, tag="r")
                u = gpool.tile([128, 2, NB], BF16, tag="u")
                inv = 1.0 / WSCALE if gate_fp8 else 1.0
                for m in range(2):
                    nc.scalar.activation(
                        r[:, m, :], pgs[m][:],
                        AF.Sigmoid, bias=bg_sb[:, m:m + 1], scale=inv,
                    )
                    nc.scalar.activation(
                        u[:, m, :], pgs[2 + m][:],
                        AF.Sigmoid, bias=bg_sb[:, 2 + m:3 + m], scale=inv,
                    )
                rh = wpool.tile([128, 2, NB], RDT, tag="rh")
                nc.vector.tensor_mul(rh[:], r[:], htb[:, :, js])
                # e2 = score*u and A = h*e2 only depend on the gate phase, so
                # they run here, off the post-tanh critical tail. A runs on
                # the otherwise-idle GPSIMD to unload the DVE.
                e2 = wpool.tile([128, 2, NB], BF16, tag="e2")
                nc.vector.tensor_mul(e2[:], u[:], grp["sbc"][:, :, js])
                A = wpool.tile([128, 2, NB], BF16, tag="A")
                a_eng = nc.gpsimd if a_pool else nc.vector
                a_eng.tensor_mul(A[:], htb[:, :, js], e2[:])
                return dict(b=b, j=j, grp=grp, rh=rh, e2=e2, A=A)

            def emit_h(st):
                """h_hat matmul + tanh + final combine + store for block b."""
                b = st["b"]
                j = st["j"]
                js = slice(j * NB, (j + 1) * NB)
                xib = st["grp"]["xib"]
                phs = [phpool.tile([128, NB], FP32, tag="ph", name=f"ph{b}_{i}")
                       for i in range(2)]
                for m in range(2):
                    for k in range(2):
                        nc.tensor.matmul(
                            phs[m][:],
                            whx_sb[:, m * 2 + k, :],
                            xib[:, k, js],
                            start=(k == 0),
                            stop=False,
                        )
                    if rh_fp8:
                        nc.tensor.matmul(
                            phs[m][:],
                            whr_sb[:, 2 * m:2 * m + 2, :],
                            st["rh"][:],
                            start=False,
                            stop=True,
                            perf_mode=DR,
                        )
                    else:
                        for k in range(2):
                            nc.tensor.matmul(
                                phs[m][:],
                                whr_sb[:, 2 * m + k, :],
                                st["rh"][:, k, :],
                                start=False,
                                stop=(k == 1),
                            )
                hhat = wpool.tile([128, 2, NB], BF16, tag="hhat")
                hsc = 1.0 / WSCALE if rh_fp8 else 1.0
                for m in range(2):
                    nc.scalar.activation(
                        hhat[:, m, :], phs[m][:],
                        AF.Tanh, bias=bh_sb[:, m:m + 1], scale=hsc,
                    )
                # out = A - (e2-1)*hh  ==  hh + e2*(h - hh), with A = h*e2
                C = wpool.tile([128, 2, NB], BF16, tag="C")
                nc.vector.scalar_tensor_tensor(
                    C[:], st["e2"][:], 1.0, hhat[:],
                    op0=mybir.AluOpType.subtract, op1=mybir.AluOpType.mult,
                )
                og = st["grp"]["og"]
                nc.vector.tensor_sub(og[:, :, js], st["A"][:], C[:])
                if j == group - 1:
                    g = st["grp"]["g"]
                    o_eng(g).dma_start(outTr[g], og[:])

            # Software-pipelined emission: block b's h-chain is emitted after
            # block b+1's gate matmuls so the PE never waits on the r*h
            # elementwise product.
            def emit_pass():
                prev = None
                for _rep in range(reps):
                    for g in range(NBLK // group):
                        grp = load_group(g)
                        for j in range(group):
                            st = emit_gates(grp, j)
                            if prev is not None:
                                emit_h(prev)
                            prev = st
                emit_h(prev)

            if loop is None:
                emit_pass()
            else:
                # bench-only: repeat the whole pass `loop` times inside one
                # NEFF execution for slope-based timing.
                with tc.For_i(0, loop, 1):
                    emit_pass()

    nc.compile()
    return nc


def _get_nc():
    if "nc" not in _NC_CACHE:
        _NC_CACHE["nc"] = _build_nc()
    return _NC_CACHE["nc"]


def _pack_weights(W_r, W_u, W_h, b_r, b_u, b_h, gate_fp8=True, rh_fp8=True,
                  swi=True):
    np8 = mybir.dt.np(FP8)
    npbf = mybir.dt.np(BF16)
    wg = np.empty((128, 24, 128), np.float32)
    for gi in range(4):
        W = W_r if gi < 2 else W_u
        m = gi % 2
        for k in range(6):
            # fp8 DoubleRow: slot gi*6 + c*2 + plane == gi*6 + k with
            # k = 2c + plane covering K rows [128k, 128k+128) — identical
            # packing for the bf16 fallback.
            wg[:, gi * 6 + k, :] = W[m * 128:(m + 1) * 128,
                                     k * 128:(k + 1) * 128].T
    if gate_fp8:
        wg = (wg * WSCALE).astype(np8)
        if swi:
            # DoubleRowSwInterleave weight layout: per chunk pair (A, B),
            # flat free order [A127, B127, A126, B126, ..., A0, B0].
            for s in range(0, 24, 2):
                A = wg[:, s, :].copy()
                Bm = wg[:, s + 1, :].copy()
                pair = np.empty((128, 256), wg.dtype)
                pair[:, 0::2] = A[:, ::-1]
                pair[:, 1::2] = Bm[:, ::-1]
                wg[:, s, :] = pair[:, :128]
                wg[:, s + 1, :] = pair[:, 128:]
    else:
        wg = wg.astype(npbf)
    hscale = WSCALE if rh_fp8 else 1.0
    whx = np.empty((128, 4, 128), np.float32)
    whr = np.empty((128, 4, 128), np.float32)
    for m in range(2):
        for k in range(2):
            whx[:, m * 2 + k, :] = W_h[m * 128:(m + 1) * 128,
                                       k * 128:(k + 1) * 128].T * hscale
            whr[:, m * 2 + k, :] = W_h[m * 128:(m + 1) * 128,
                                       (2 + k) * 128:(3 + k) * 128].T * hscale
    whx = whx.astype(npbf)
    whr = whr.astype(np8 if rh_fp8 else npbf)
    bg = np.stack([b_r[:128], b_r[128:], b_u[:128], b_u[128:]], axis=1)
    bh = np.stack([b_h[:128], b_h[128:]], axis=1)
    return (np.ascontiguousarray(wg), np.ascontiguousarray(whx),
            np.ascontiguousarray(whr),
            np.ascontiguousarray(bg), np.ascontiguousarray(bh))


def _make_in_maps(inputs, h_prev, attention_score, W_r, b_r, W_u, b_u,
                  W_h, b_h, gate_fp8=True, rh_fp8=False, swi=True):
    np8 = mybir.dt.np(FP8)
    npbf = mybir.dt.np(BF16)
    gdt = np8 if gate_fp8 else npbf
    inputs = np.asarray(inputs, np.float32)
    h_prev = np.asarray(h_prev, np.float32)
    attention_score = np.asarray(attention_score, np.float32)
    wg, whx, whr, bg, bh = _pack_weights(
        np.asarray(W_r, np.float32), np.asarray(W_u, np.float32),
        np.asarray(W_h, np.float32), np.asarray(b_r, np.float32),
        np.asarray(b_u, np.float32), np.asarray(b_h, np.float32),
        gate_fp8=gate_fp8, rh_fp8=rh_fp8, swi=swi,
    )
    in_maps = []
    for c in range(NCORES):
        sl = slice(c * BC, (c + 1) * BC)
        xT = np.ascontiguousarray(inputs[sl].T)
        hT = np.ascontiguousarray(h_prev[sl].T)
        in_maps.append({
            "x8": xT.astype(gdt),
            "h8": hT.astype(gdt),
            "xi": np.ascontiguousarray(xT[:I]).astype(npbf),
            "hb": hT.astype(npbf),
            "sc": np.ascontiguousarray(
                attention_score[sl].reshape(NBLK, 1, NB)).astype(npbf),
            "wg": wg, "whx": whx, "whr": whr, "bg": bg, "bh": bh,
        })
    return in_maps


def _run(in_maps, trace=False, **kwargs):
    try:
        return run_bass_kernel_spmd(
            _get_nc(), in_maps, core_ids=list(range(NCORES)), trace=trace, **kwargs
        )
    except ModuleNotFoundError:
        # A global BASS_TRACE=1 enables the NTFF trace path, which needs
        # antenv.axon_hooks; on images without it, retry untraced.
        had = os.environ.get("BASS_NEVER_TRACE")
        os.environ["BASS_NEVER_TRACE"] = "1"
        try:
            return run_bass_kernel_spmd(
                _get_nc(), in_maps, core_ids=list(range(NCORES)), trace=False,
                **kwargs
            )
        finally:
            if had is None:
                del os.environ["BASS_NEVER_TRACE"]
            else:
                os.environ["BASS_NEVER_TRACE"] = had


def _gather(results):
    out = np.empty((B, H), np.float32)
    for c in range(NCORES):
        out[c * BC:(c + 1) * BC] = results[c]["outT"].T.astype(np.float32)
    return out


def kernel(**inputs):
    res = _run(_make_in_maps(**inputs), trace=False)
    return _gather(res.results)
